# revision 5
# baseline (speedup 1.0000x reference)
"""Trainium2 Bass kernel for nn_LogicLayer (difflogic soft-logic layer).

Math: out[i, j] = c0[j] + ca[j]*a + cb[j]*b + cab[j]*a*b
  where a = x[i, idx_a[j]], b = x[i, idx_b[j]] and the c* coefficients are
  linear combinations of softmax(weights[j]) (all 16 soft logic gates are
  polynomials over the basis {1, a, b, a*b}):
    E_g = exp(w_g) / sum_g exp(w_g)
    c0  = E8+E9+E10+E11+E12+E13+E14+E15
    ca  = E2+E3+E6+E7-E8-E9-E12-E13
    cb  = E4+E5+E6+E7-E8-E9-E10-E11
    cab = E1-E2-E4-2*E6-E7+E8+2*E9+E11+E13-E14

Sharding: output neurons are split across the 8 NeuronCores (2048 each).
Per core the kernel works in a transposed layout (neuron on the partition
axis, batch on the free axis): `dma_gather` pulls rows of xT = x.T from HBM,
landing row j at partition j%128, so the per-neuron coefficients become
per-partition scalars.

fp16 everywhere on the data path: xT is downconverted to fp16 on the host
(halves the gather read: 16MB/core), the output tile is computed and stored
as fp16 (halves the store: 8MB/core), and the elementwise ops run in DVE's
2x packed-16-bit mode. x is in [0,1) and the softmax-derived coefficients
are O(1), so fp16 keeps rel err ~1e-3, well under the 2e-2 gate.
"""
import numpy as np

import concourse.bacc as bacc
import concourse.mybir as mybir
import concourse.tile as tile
from concourse import bass_utils

F32 = mybir.dt.float32
F16 = mybir.dt.float16
I16 = mybir.dt.int16

B = 2048
IN_DIM = 8192
OUT_DIM = 16384
NCORES = 8
OUTC = OUT_DIM // NCORES     # neurons per core
NT = OUTC // 128             # 128-neuron tiles per core
IDX_PER = 512                # indices per dma_gather call (2 tiles x (a,b))
NQ = 4                       # SWDGE queues

_CACHE = {}


def _build_nc(repeats=1, mode="full"):
    """mode: 'full' | 'dma' (gather+store, no compute) | 'compute' (no gather)
    | 'gather' (gather only) | 'store' (store only)."""
    nc = bacc.Bacc("TRN2", target_bir_lowering=False, debug=False,
                   enable_asserts=False, num_swdge_queues=NQ)
    xT_d = nc.dram_tensor("xT", [IN_DIM, B], F16, kind="ExternalInput")
    w_d = nc.dram_tensor("wre", [128, NT * 16], F32, kind="ExternalInput")
    idx_d = nc.dram_tensor("idxw", [128, NT * 16], I16, kind="ExternalInput")
    out_d = nc.dram_tensor("outT", [OUTC, B], F16, kind="ExternalOutput")

    add = mybir.AluOpType.add
    sub = mybir.AluOpType.subtract
    mult = mybir.AluOpType.mult
    ident = mybir.ActivationFunctionType.Identity

    with tile.TileContext(nc) as tc:
        with (
            tc.tile_pool(name="const", bufs=1) as cpool,
            tc.tile_pool(name="gat", bufs=3) as gpool,
            tc.tile_pool(name="uv", bufs=3) as uvpool,
            tc.tile_pool(name="out", bufs=3) as opool,
        ):
            idx_sb = cpool.tile([128, NT * 16], I16, tag="idx")
            nc.sync.dma_start(idx_sb[:], idx_d.ap())
            w_sb = cpool.tile([128, NT * 16], F32, tag="w")
            nc.sync.dma_start(w_sb[:], w_d.ap())

            # ---- per-neuron coefficients from softmax(weights) ----
            # weights ~ N(0,1): exp() cannot overflow f32, skip max-subtraction
            e = cpool.tile([128, NT * 16], F32, tag="e")
            nc.scalar.activation(e[:], w_sb[:], mybir.ActivationFunctionType.Exp)
            e3 = e[:].rearrange("p (t g) -> p t g", g=16)

            s = cpool.tile([128, NT], F32, tag="s")
            nc.vector.tensor_reduce(s[:], e3, mybir.AxisListType.X, add)
            r = cpool.tile([128, NT], F32, tag="r")
            nc.vector.reciprocal(r[:], s[:])

            def E(g):
                return e3[:, :, g]

            def tt(out, a_, b_, op):
                nc.vector.tensor_tensor(out, a_, b_, op)

            s67 = cpool.tile([128, NT], F32, tag="s67")
            tt(s67[:], E(6), E(7), add)
            s89 = cpool.tile([128, NT], F32, tag="s89")
            tt(s89[:], E(8), E(9), add)
            s1011 = cpool.tile([128, NT], F32, tag="s1011")
            tt(s1011[:], E(10), E(11), add)
            s1213 = cpool.tile([128, NT], F32, tag="s1213")
            tt(s1213[:], E(12), E(13), add)

            c0 = cpool.tile([128, NT], F32, tag="c0")
            tt(c0[:], s89[:], s1011[:], add)
            tt(c0[:], c0[:], s1213[:], add)
            tt(c0[:], c0[:], E(14), add)
            tt(c0[:], c0[:], E(15), add)

            ca = cpool.tile([128, NT], F32, tag="ca")
            tt(ca[:], E(2), E(3), add)
            tt(ca[:], ca[:], s67[:], add)
            tt(ca[:], ca[:], s89[:], sub)
            tt(ca[:], ca[:], s1213[:], sub)

            cb = cpool.tile([128, NT], F32, tag="cb")
            tt(cb[:], E(4), E(5), add)
            tt(cb[:], cb[:], s67[:], add)
            tt(cb[:], cb[:], s89[:], sub)
            tt(cb[:], cb[:], s1011[:], sub)

            cab = cpool.tile([128, NT], F32, tag="cab")
            tt(cab[:], E(1), E(2), sub)
            tt(cab[:], cab[:], E(4), sub)
            tt(cab[:], cab[:], s67[:], sub)
            tt(cab[:], cab[:], E(6), sub)
            tt(cab[:], cab[:], s89[:], add)
            tt(cab[:], cab[:], E(9), add)
            tt(cab[:], cab[:], E(11), add)
            tt(cab[:], cab[:], E(13), add)
            tt(cab[:], cab[:], E(14), sub)

            for t_ in (c0, ca, cb, cab):
                tt(t_[:], t_[:], r[:], mult)

            # ---- gather + compute + store ----
            tiles_per_call = IDX_PER // 256
            g0 = None
            if mode in ("compute", "store"):
                g0 = cpool.tile([128, tiles_per_call * 2, B], F16, tag="g0")
                nc.vector.memset(g0[:], 0.5)
            for rep in range(repeats):
                for call in range(NT // tiles_per_call):
                    if mode in ("full", "dma", "gather"):
                        g = gpool.tile([128, tiles_per_call * 2, B], F16, tag="g")
                        nc.gpsimd.dma_gather(
                            g[:], xT_d.ap(),
                            idx_sb[:, call * 16 * tiles_per_call:
                                   (call + 1) * 16 * tiles_per_call],
                            IDX_PER, IDX_PER, B,
                            queue_num=call % NQ, single_packet=False,
                        )
                    else:
                        g = g0
                    for k in range(tiles_per_call):
                        t = call * tiles_per_call + k
                        a_ap = g[:, 2 * k, :]
                        b_ap = g[:, 2 * k + 1, :]
                        if mode == "gather":
                            continue
                        if mode in ("dma", "store"):
                            nc.sync.dma_start(
                                out_d.ap()[t * 128:(t + 1) * 128, :], a_ap)
                            continue
                        # u = cab*b + ca  (ScalarE, per-partition scalars)
                        u = uvpool.tile([128, B], F16, tag="u")
                        nc.scalar.activation(u[:], b_ap, ident,
                                             bias=ca[:, t:t + 1],
                                             scale=cab[:, t:t + 1])
                        # v = cb*b + c0   (VectorE fused tensor_scalar)
                        v = uvpool.tile([128, B], F16, tag="v")
                        nc.vector.tensor_scalar(v[:], b_ap,
                                                cb[:, t:t + 1], c0[:, t:t + 1],
                                                mult, add)
                        w_ = opool.tile([128, B], F16, tag="wk")
                        tt(w_[:], a_ap, u[:], mult)
                        o = opool.tile([128, B], F16, tag="o")
                        tt(o[:], w_[:], v[:], add)
                        nc.sync.dma_start(out_d.ap()[t * 128:(t + 1) * 128, :], o[:])

    nc.compile()
    return nc


def _wrap_idxs(idx):
    """[n] -> [128, n//16] int16: wrapped[p, s] = idx[s*16 + p%16]."""
    n = idx.shape[0]
    w16 = idx.reshape(n // 16, 16).T.astype(np.int16)
    return np.tile(w16, (8, 1))


def _host_prep(x, weights, idx_a, idx_b):
    xT = np.ascontiguousarray(np.asarray(x, dtype=np.float32).T.astype(np.float16))
    weights = np.asarray(weights, dtype=np.float32)
    idx_a = np.asarray(idx_a)
    idx_b = np.asarray(idx_b)
    in_maps = []
    for c in range(NCORES):
        lo = c * OUTC
        ia = idx_a[lo:lo + OUTC]
        ib = idx_b[lo:lo + OUTC]
        cols = []
        for t in range(NT):
            seq = np.concatenate([ia[t * 128:(t + 1) * 128],
                                  ib[t * 128:(t + 1) * 128]])
            cols.append(_wrap_idxs(seq))
        idxw = np.ascontiguousarray(np.concatenate(cols, axis=1))
        wc = weights[lo:lo + OUTC]
        wre = np.ascontiguousarray(
            wc.reshape(NT, 128, 16).transpose(1, 0, 2).reshape(128, NT * 16))
        in_maps.append({"xT": xT, "wre": wre, "idxw": idxw})
    return in_maps


def kernel(x, weights, idx_a, idx_b):
    x = np.asarray(x)
    out_dtype = x.dtype
    if "nc" not in _CACHE:
        _CACHE["nc"] = _build_nc()
    nc = _CACHE["nc"]

    in_maps = _host_prep(x, weights, idx_a, idx_b)
    res = bass_utils.run_bass_kernel_spmd(nc, in_maps,
                                          core_ids=list(range(NCORES)))
    out = np.empty((B, OUT_DIM), dtype=out_dtype)
    for c in range(NCORES):
        out[:, c * OUTC:(c + 1) * OUTC] = res.results[c]["outT"].T
    return out


# revision 7
# speedup vs baseline: 1.9568x; 1.9568x over previous
"""Trainium2 Bass kernel for nn_LogicLayer (difflogic soft-logic layer).

Math: out[i, j] = c0[j] + ca[j]*a + cb[j]*b + cab[j]*a*b
  where a = x[i, idx_a[j]], b = x[i, idx_b[j]] and the c* coefficients are
  linear combinations of softmax(weights[j]) (all 16 soft logic gates are
  polynomials over the basis {1, a, b, a*b}):
    E_g = exp(w_g) / sum_g exp(w_g)
    c0  = E8+E9+E10+E11+E12+E13+E14+E15
    ca  = E2+E3+E6+E7-E8-E9-E12-E13
    cb  = E4+E5+E6+E7-E8-E9-E10-E11
    cab = E1-E2-E4-2*E6-E7+E8+2*E9+E11+E13-E14

Sharding: output neurons are split across the 8 NeuronCores (2048 each).
Per core the kernel works in a transposed layout (neuron on the partition
axis, batch on the free axis): `dma_gather` pulls rows of xT = x.T from HBM,
landing row j at partition j%128, so the per-neuron coefficients become
per-partition scalars.

fp16 everywhere on the data path: xT is downconverted to fp16 on the host
(halves the gather read: 16MB/core), the output tile is computed and stored
as fp16 (halves the store: 8MB/core), and the elementwise ops run in DVE's
2x packed-16-bit mode. x is in [0,1) and the softmax-derived coefficients
are O(1), so fp16 keeps rel err ~1e-3, well under the 2e-2 gate.
"""
import numpy as np

import concourse.bacc as bacc
import concourse.mybir as mybir
import concourse.tile as tile
from concourse import bass_utils

F32 = mybir.dt.float32
F16 = mybir.dt.float16
I16 = mybir.dt.int16

B = 2048
IN_DIM = 8192
OUT_DIM = 16384
NCORES = 8
OUTC = OUT_DIM // NCORES     # neurons per core
NT = OUTC // 128             # 128-neuron tiles per core
IDX_PER = 512                # indices per dma_gather call (2 tiles x (a,b))
NQ = 4                       # SWDGE queues

_CACHE = {}


def _build_nc(repeats=1, mode="full", nq=NQ, idx_per=IDX_PER, coalesce=1):
    """mode: 'full' | 'dma' (gather+store, no compute) | 'compute' (no gather)
    | 'gather' (gather only) | 'store' (store only)."""
    nc = bacc.Bacc("TRN2", target_bir_lowering=False, debug=False,
                   enable_asserts=False, num_swdge_queues=nq)
    xT_d = nc.dram_tensor("xT", [IN_DIM, B], F16, kind="ExternalInput")
    w_d = nc.dram_tensor("wre", [128, NT * 16], F32, kind="ExternalInput")
    idx_d = nc.dram_tensor("idxw", [128, NT * 16], I16, kind="ExternalInput")
    out_d = nc.dram_tensor("outT", [OUTC, B], F16, kind="ExternalOutput")

    add = mybir.AluOpType.add
    sub = mybir.AluOpType.subtract
    mult = mybir.AluOpType.mult
    ident = mybir.ActivationFunctionType.Identity

    with tile.TileContext(nc) as tc:
        with (
            tc.tile_pool(name="const", bufs=1) as cpool,
            tc.tile_pool(name="gat", bufs=3) as gpool,
            tc.tile_pool(name="uv", bufs=3) as uvpool,
            tc.tile_pool(name="out", bufs=3) as opool,
        ):
            idx_sb = cpool.tile([128, NT * 16], I16, tag="idx")
            nc.sync.dma_start(idx_sb[:], idx_d.ap())
            w_sb = cpool.tile([128, NT * 16], F32, tag="w")
            nc.sync.dma_start(w_sb[:], w_d.ap())

            # ---- per-neuron coefficients from softmax(weights) ----
            # weights ~ N(0,1): exp() cannot overflow f32, skip max-subtraction
            e = cpool.tile([128, NT * 16], F32, tag="e")
            nc.scalar.activation(e[:], w_sb[:], mybir.ActivationFunctionType.Exp)
            e3 = e[:].rearrange("p (t g) -> p t g", g=16)

            s = cpool.tile([128, NT], F32, tag="s")
            nc.vector.tensor_reduce(s[:], e3, mybir.AxisListType.X, add)
            r = cpool.tile([128, NT], F32, tag="r")
            nc.vector.reciprocal(r[:], s[:])

            def E(g):
                return e3[:, :, g]

            def tt(out, a_, b_, op):
                nc.vector.tensor_tensor(out, a_, b_, op)

            s67 = cpool.tile([128, NT], F32, tag="s67")
            tt(s67[:], E(6), E(7), add)
            s89 = cpool.tile([128, NT], F32, tag="s89")
            tt(s89[:], E(8), E(9), add)
            s1011 = cpool.tile([128, NT], F32, tag="s1011")
            tt(s1011[:], E(10), E(11), add)
            s1213 = cpool.tile([128, NT], F32, tag="s1213")
            tt(s1213[:], E(12), E(13), add)

            c0 = cpool.tile([128, NT], F32, tag="c0")
            tt(c0[:], s89[:], s1011[:], add)
            tt(c0[:], c0[:], s1213[:], add)
            tt(c0[:], c0[:], E(14), add)
            tt(c0[:], c0[:], E(15), add)

            ca = cpool.tile([128, NT], F32, tag="ca")
            tt(ca[:], E(2), E(3), add)
            tt(ca[:], ca[:], s67[:], add)
            tt(ca[:], ca[:], s89[:], sub)
            tt(ca[:], ca[:], s1213[:], sub)

            cb = cpool.tile([128, NT], F32, tag="cb")
            tt(cb[:], E(4), E(5), add)
            tt(cb[:], cb[:], s67[:], add)
            tt(cb[:], cb[:], s89[:], sub)
            tt(cb[:], cb[:], s1011[:], sub)

            cab = cpool.tile([128, NT], F32, tag="cab")
            tt(cab[:], E(1), E(2), sub)
            tt(cab[:], cab[:], E(4), sub)
            tt(cab[:], cab[:], s67[:], sub)
            tt(cab[:], cab[:], E(6), sub)
            tt(cab[:], cab[:], s89[:], add)
            tt(cab[:], cab[:], E(9), add)
            tt(cab[:], cab[:], E(11), add)
            tt(cab[:], cab[:], E(13), add)
            tt(cab[:], cab[:], E(14), sub)

            for t_ in (c0, ca, cb, cab):
                tt(t_[:], t_[:], r[:], mult)

            # ---- gather + compute + store ----
            tiles_per_call = idx_per // 256
            g0 = None
            if mode in ("compute", "store"):
                g0 = cpool.tile([128, tiles_per_call * 2, B], F16, tag="g0")
                nc.vector.memset(g0[:], 0.5)
            for rep in range(repeats):
                for call in range(NT // tiles_per_call):
                    if mode in ("full", "dma", "gather"):
                        g = gpool.tile([128, tiles_per_call * 2, B], F16, tag="g")
                        nc.gpsimd.dma_gather(
                            g[:], xT_d.ap(),
                            idx_sb[:, call * 16 * tiles_per_call:
                                   (call + 1) * 16 * tiles_per_call],
                            idx_per, idx_per, B,
                            queue_num=call % nq, single_packet=False,
                        )
                    else:
                        g = g0
                    ob = None
                    if mode == "full" and coalesce > 1:
                        ob = opool.tile([128, coalesce, B], F16, tag="ob")
                    for k in range(tiles_per_call):
                        t = call * tiles_per_call + k
                        a_ap = g[:, 2 * k, :]
                        b_ap = g[:, 2 * k + 1, :]
                        if mode == "gather":
                            continue
                        if mode in ("dma", "store"):
                            nc.sync.dma_start(
                                out_d.ap()[t * 128:(t + 1) * 128, :], a_ap)
                            continue
                        # u = cab*b + ca  (ScalarE, per-partition scalars)
                        u = uvpool.tile([128, B], F16, tag="u")
                        nc.scalar.activation(u[:], b_ap, ident,
                                             bias=ca[:, t:t + 1],
                                             scale=cab[:, t:t + 1])
                        # v = cb*b + c0   (VectorE fused tensor_scalar)
                        v = uvpool.tile([128, B], F16, tag="v")
                        nc.vector.tensor_scalar(v[:], b_ap,
                                                cb[:, t:t + 1], c0[:, t:t + 1],
                                                mult, add)
                        w_ = opool.tile([128, B], F16, tag="wk")
                        tt(w_[:], a_ap, u[:], mult)
                        if coalesce > 1:
                            kc = k % coalesce
                            tt(ob[:, kc, :], w_[:], v[:], add)
                            if kc == coalesce - 1:
                                t0 = t - coalesce + 1
                                dst = out_d.ap()[t0 * 128:(t0 + coalesce) * 128, :]
                                dst = dst.rearrange("(j p) c -> p j c", p=128)
                                nc.sync.dma_start(dst, ob[:])
                                if k + 1 < tiles_per_call:
                                    ob = opool.tile([128, coalesce, B], F16,
                                                    tag="ob")
                        else:
                            o = opool.tile([128, B], F16, tag="o")
                            tt(o[:], w_[:], v[:], add)
                            nc.sync.dma_start(
                                out_d.ap()[t * 128:(t + 1) * 128, :], o[:])

    nc.compile()
    return nc


def _wrap_idxs(idx):
    """[n] -> [128, n//16] int16: wrapped[p, s] = idx[s*16 + p%16]."""
    n = idx.shape[0]
    w16 = idx.reshape(n // 16, 16).T.astype(np.int16)
    return np.tile(w16, (8, 1))


def _host_prep(x, weights, idx_a, idx_b):
    xT = np.ascontiguousarray(np.asarray(x, dtype=np.float32).T.astype(np.float16))
    weights = np.asarray(weights, dtype=np.float32)
    idx_a = np.asarray(idx_a)
    idx_b = np.asarray(idx_b)
    in_maps = []
    for c in range(NCORES):
        lo = c * OUTC
        ia = idx_a[lo:lo + OUTC]
        ib = idx_b[lo:lo + OUTC]
        cols = []
        for t in range(NT):
            seq = np.concatenate([ia[t * 128:(t + 1) * 128],
                                  ib[t * 128:(t + 1) * 128]])
            cols.append(_wrap_idxs(seq))
        idxw = np.ascontiguousarray(np.concatenate(cols, axis=1))
        wc = weights[lo:lo + OUTC]
        wre = np.ascontiguousarray(
            wc.reshape(NT, 128, 16).transpose(1, 0, 2).reshape(128, NT * 16))
        in_maps.append({"xT": xT, "wre": wre, "idxw": idxw})
    return in_maps


def kernel(x, weights, idx_a, idx_b):
    x = np.asarray(x)
    out_dtype = x.dtype
    if "nc" not in _CACHE:
        _CACHE["nc"] = _build_nc()
    nc = _CACHE["nc"]

    in_maps = _host_prep(x, weights, idx_a, idx_b)
    res = bass_utils.run_bass_kernel_spmd(nc, in_maps,
                                          core_ids=list(range(NCORES)))
    out = np.empty((B, OUT_DIM), dtype=out_dtype)
    for c in range(NCORES):
        out[:, c * OUTC:(c + 1) * OUTC] = res.results[c]["outT"].T
    return out
